# revision 2
# baseline (speedup 1.0000x reference)
"""Trainium2 Bass kernel for 2-layer GATv2 (nn_EvenLamerGAT).

Strategy (8 NeuronCores, SPMD single launch):
  - Host: append self-loops, sort edges by dst, partition dst nodes into 8
    contiguous ranges with ~equal edge counts. Each core owns one dst range.
  - Per core: compute xl/xr node transforms for its node shard (PE matmuls),
    AllGather the xl shards (bf16) into a replicated table, then process its
    edges in dst-blocks of 128 nodes: dma_gather the xl rows per edge,
    build one-hot incidence matrices from the local dst ids, and do the
    segment softmax + aggregation entirely with PE matmuls.
  - Layer 2 repeats the pattern with the layer-1 output (one AllGather of the
    xl2 shards), then log_softmax.

All schedule shapes (tile counts, paddings) are derived from the actual
edge_index passed to kernel(); the Bass program is compiled per call.
"""
import os
import sys

sys.path.insert(0, "/opt/trn_rl_repo")

import numpy as np
import ml_dtypes

from concourse import bass, mybir, bacc, tile
from concourse import bass_utils

F32 = mybir.dt.float32
BF16 = mybir.dt.bfloat16
I16 = mybir.dt.int16
AF = mybir.ActivationFunctionType
OP = mybir.AluOpType

NCORES = 8
SPLIT = 32768          # int16 gather index limit
NEG_SLOPE = 0.2
H, C = 8, 32
HC = H * C             # 256
DIN = 128
DOUT = 64
GATHER_TILES_PER_CALL = 8   # 1024 idxs per dma_gather (hard per-call cap)
NQ = 4                 # SWDGE queues


def _wrap_idx16(idx, num):
    """Wrap `num` int16 indices into the [128, num//16] dma_gather layout."""
    assert num % 128 == 0 and len(idx) == num
    w = np.zeros((128, num // 16), np.int16)
    blk = idx.reshape(num // 16, 16).T
    for g in range(8):
        w[g * 16:(g + 1) * 16, :] = blk
    return w


def _prep_host(x, edge_index):
    N = x.shape[0]
    src = np.concatenate([edge_index[0], np.arange(N, dtype=np.int64)]).astype(np.int64)
    dst = np.concatenate([edge_index[1], np.arange(N, dtype=np.int64)]).astype(np.int64)
    order = np.argsort(dst, kind="stable")
    src_s = src[order].astype(np.int64)
    dst_s = dst[order].astype(np.int64)
    Etot = len(src_s)

    # core ranges: contiguous node spans with ~equal edge counts
    deg = np.bincount(dst_s, minlength=N)
    cum = np.cumsum(deg)
    starts = [0]
    for k in range(1, NCORES):
        starts.append(int(np.searchsorted(cum, k * Etot / NCORES)))
    starts.append(N)
    starts = np.array(starts, np.int64)
    counts = starts[1:] - starts[:-1]
    Np = int(np.ceil(counts.max() / 128) * 128)
    NB = Np // 128
    assert NCORES * Np < 2 * SPLIT, "row index must fit int16 after lo/hi split"

    # global node -> replicated-table row
    owner = np.searchsorted(starts[1:], np.arange(N), side="right")
    table_row = owner * Np + (np.arange(N) - starts[owner])
    src_row = table_row[src_s]
    edge_start = np.searchsorted(dst_s, starts[:-1])
    edge_end = np.searchsorted(dst_s, starts[1:])

    # First pass: per (core, block) lo/hi edge lists (row, dstl)
    per_block = []  # [core][block] = (lo_rows, lo_dstl, hi_rows, hi_dstl)
    T_LO = T_HI = 1
    for c in range(NCORES):
        s0, n_c = starts[c], counts[c]
        blocks = []
        e0, e1 = edge_start[c], edge_end[c]
        er = src_row[e0:e1]
        ed = dst_s[e0:e1] - s0            # local dst 0..n_c-1
        for b in range(NB):
            lo_d, hi_d = b * 128, (b + 1) * 128
            m = (ed >= lo_d) & (ed < hi_d)
            rows = er[m]
            dl = (ed[m] - lo_d).astype(np.float32)
            lo = rows < SPLIT
            lo_rows = rows[lo].astype(np.int64)
            lo_dstl = dl[lo]
            hi_rows = rows[~lo] - SPLIT
            hi_dstl = dl[~lo]
            # dummy edges so padded dst slots have nonzero denominators
            nreal = max(0, min(128, n_c - lo_d))
            if nreal < 128:
                pad_d = np.arange(nreal, 128, dtype=np.float32)
                lo_rows = np.concatenate([lo_rows, np.zeros(len(pad_d), np.int64)])
                lo_dstl = np.concatenate([lo_dstl, pad_d])
            blocks.append((lo_rows, lo_dstl, hi_rows, hi_dstl))
            T_LO = max(T_LO, (len(lo_rows) + 127) // 128)
            T_HI = max(T_HI, (len(hi_rows) + 127) // 128)
        per_block.append(blocks)

    T = T_LO + T_HI
    # Second pass: fill padded arrays
    gidx = np.zeros((NCORES, 128, NB * T * 8), np.int16)
    dstl = np.full((NCORES, 128, NB * T), 300.0, np.float32)
    for c in range(NCORES):
        for b in range(NB):
            lo_rows, lo_dstl, hi_rows, hi_dstl = per_block[c][b]
            for (rows, dls, toff, nt) in (
                (lo_rows, lo_dstl, 0, T_LO),
                (hi_rows, hi_dstl, T_LO, T_HI),
            ):
                n = nt * 128
                ridx = np.zeros(n, np.int64)
                ridx[: len(rows)] = rows
                dpad = np.full(n, 300.0, np.float32)
                dpad[: len(dls)] = dls
                # dstl layout: slot k=(t*128+p) -> [p, b*T+toff+t]
                dstl[c][:, b * T + toff:b * T + toff + nt] = (
                    dpad.reshape(nt, 128).T
                )
                # gather idx layout: wrapped per call chunk
                for t0 in range(0, nt, GATHER_TILES_PER_CALL):
                    ntc = min(GATHER_TILES_PER_CALL, nt - t0)
                    chunk = ridx[t0 * 128:(t0 + ntc) * 128].astype(np.int16)
                    col0 = (b * T + toff + t0) * 8
                    gidx[c][:, col0:col0 + ntc * 8] = _wrap_idx16(chunk, ntc * 128)

    xT = np.ascontiguousarray(x.T)  # [128, N]
    xT_sh = np.zeros((NCORES, DIN, Np), np.float32)
    for c in range(NCORES):
        xT_sh[c, :, : counts[c]] = xT[:, starts[c]:starts[c] + counts[c]]

    return dict(N=N, starts=starts, counts=counts, Np=Np, NB=NB,
                T_LO=T_LO, T_HI=T_HI, T=T, gidx=gidx, dstl=dstl, xT_sh=xT_sh)


def _build_bass(P):
    """Build the SPMD Bass program for prep dict P."""
    Np, NB, T_LO, T_HI, T = P["Np"], P["NB"], P["T_LO"], P["T_HI"], P["T"]
    NROWS = NCORES * Np

    nc = bacc.Bacc("TRN2", target_bir_lowering=False, debug=False,
                   enable_asserts=True, num_devices=NCORES, num_swdge_queues=NQ)

    din = lambda n, s, d: nc.dram_tensor(n, s, d, kind="ExternalInput").ap()
    xT_in = din("xT", [DIN, Np], F32)
    gidx_in = din("gidx", [128, NB * T * 8], I16)
    dstl_in = din("dstl", [128, NB * T], F32)
    iota_in = din("iota", [128, 128], F32)
    identb_in = din("identb", [128, 128], BF16)
    identf_in = din("identf", [128, 128], F32)
    att1_in = din("att1r", [128, HC], F32)
    att2_in = din("att2r", [128, DOUT], F32)
    w1l_in = din("w1l", [DIN, HC], F32)
    w1r_in = din("w1r", [DIN, HC], F32)
    w2l_in = din("w2l", [HC, DOUT], F32)
    w2r_in = din("w2r", [HC, DOUT], F32)
    bl1_in = din("bl1r", [128, HC], F32)
    br1_in = din("br1r", [128, HC], F32)
    b1_in = din("b1r", [128, HC], F32)
    bl2_in = din("bl2r", [128, DOUT], F32)
    br2_in = din("br2r", [128, DOUT], F32)
    b2_in = din("b2r", [128, DOUT], F32)
    out1 = nc.dram_tensor("out1", [Np, DOUT], F32, kind="ExternalOutput").ap()
    out2 = nc.dram_tensor("out2", [Np, DOUT], F32, kind="ExternalOutput").ap()

    qctr = [0]

    def next_q():
        q = qctr[0] % NQ
        qctr[0] += 1
        return q

    with tile.TileContext(nc) as tc:
        import contextlib
        with contextlib.ExitStack() as ctx:
            cn = ctx.enter_context(tc.tile_pool(name="const", bufs=1))
            dr = ctx.enter_context(tc.tile_pool(name="dram", bufs=1, space="DRAM"))

            def load_const(ap_in, shape, dt, cast=False):
                t = cn.tile(shape, dt, tag=ap_in.tensor.name)
                (nc.gpsimd if cast else nc.sync).dma_start(out=t[:], in_=ap_in[:])
                return t

            iota = load_const(iota_in, [128, 128], F32)
            identb = load_const(identb_in, [128, 128], BF16)
            identf = load_const(identf_in, [128, 128], F32)
            att1b = load_const(att1_in, [128, HC], BF16, cast=True)
            att2f = load_const(att2_in, [128, DOUT], F32)
            w1l = load_const(w1l_in, [DIN, HC], F32)
            w1r = load_const(w1r_in, [DIN, HC], F32)
            bl1 = load_const(bl1_in, [128, HC], F32)
            br1 = load_const(br1_in, [128, HC], F32)
            b1r = load_const(b1_in, [128, HC], F32)
            bl2 = load_const(bl2_in, [128, DOUT], F32)
            br2 = load_const(br2_in, [128, DOUT], F32)
            b2r = load_const(b2_in, [128, DOUT], F32)
            # W2 as [128, 2, DOUT] bf16 (rows 0:128, 128:256)
            w2lb = cn.tile([128, 2, DOUT], BF16)
            nc.gpsimd.dma_start(out=w2lb[:, 0, :], in_=w2l_in[0:128, :])
            nc.gpsimd.dma_start(out=w2lb[:, 1, :], in_=w2l_in[128:256, :])
            w2rb = cn.tile([128, 2, DOUT], BF16)
            nc.gpsimd.dma_start(out=w2rb[:, 0, :], in_=w2r_in[0:128, :])
            nc.gpsimd.dma_start(out=w2rb[:, 1, :], in_=w2r_in[128:256, :])
            gidx_sb = cn.tile([128, NB * T * 8], I16)
            nc.sync.dma_start(out=gidx_sb[:], in_=gidx_in[:])
            dstl_sb = cn.tile([128, NB * T], F32)
            nc.sync.dma_start(out=dstl_sb[:], in_=dstl_in[:])

            # DRAM buffers
            xl_ag_in = dr.tile([Np, HC], BF16)
            xl_full = dr.tile([NROWS, HC], BF16, addr_space="Shared")
            xl2_ag_in = dr.tile([Np, DOUT], F32)
            xl2_full = dr.tile([NROWS, DOUT], F32, addr_space="Shared")

            persist1 = ctx.enter_context(tc.tile_pool(name="persist1", bufs=1))
            xr_sh = persist1.tile([128, NB, HC], BF16)
            h_sh = persist1.tile([128, NB, HC], F32)
            persist2 = ctx.enter_context(tc.tile_pool(name="persist2", bufs=1))
            xr2_sh = persist2.tile([128, NB, DOUT], F32)
            o1_sh = persist2.tile([128, NB, DOUT], F32)

            # ---- Phase A: xl/xr shard transforms ----
            with (
                tc.tile_pool(name="pa_sb", bufs=3) as pa,
                tc.tile_pool(name="pa_ps", bufs=2, space="PSUM") as pap,
            ):
                for i in range(NB):
                    xt = pa.tile([128, 128], F32, tag="xt")
                    nc.sync.dma_start(out=xt[:], in_=xT_in[:, i * 128:(i + 1) * 128])
                    psl = pap.tile([128, HC], F32, space="PSUM", tag="psl")
                    nc.tensor.matmul(out=psl[:], lhsT=xt[:], rhs=w1l[:], start=True, stop=True)
                    xlt = pa.tile([128, HC], BF16, tag="xlt")
                    nc.vector.tensor_tensor(out=xlt[:], in0=psl[:], in1=bl1[:], op=OP.add)
                    nc.sync.dma_start(out=xl_ag_in[i * 128:(i + 1) * 128, :], in_=xlt[:])
                    psr = pap.tile([128, HC], F32, space="PSUM", tag="psr")
                    nc.tensor.matmul(out=psr[:], lhsT=xt[:], rhs=w1r[:], start=True, stop=True)
                    nc.vector.tensor_tensor(out=xr_sh[:, i, :], in0=psr[:], in1=br1[:], op=OP.add)

            nc.gpsimd.collective_compute(
                "AllGather", OP.bypass, replica_groups=[list(range(NCORES))],
                ins=[xl_ag_in[:].opt()], outs=[xl_full[:].opt()],
            )

            # ---- Phase B: layer-1 edge loop ----
            with (
                tc.tile_pool(name="pb_sb", bufs=2) as pb,
                tc.tile_pool(name="pb_s", bufs=T + 2) as pbs,
                tc.tile_pool(name="pb_ps", bufs=2, space="PSUM") as pbp,
                tc.tile_pool(name="pb_acc", bufs=2, space="PSUM") as pba,
            ):
                for b in range(NB):
                    xlg = pb.tile([128, T, HC], BF16, tag="xlg")
                    for (toff, nt, base) in ((0, T_LO, 0), (T_LO, T_HI, SPLIT)):
                        src_ap = xl_full[:] if base == 0 else xl_full[base:, :]
                        for t0 in range(0, nt, GATHER_TILES_PER_CALL):
                            ntc = min(GATHER_TILES_PER_CALL, nt - t0)
                            col0 = (b * T + toff + t0) * 8
                            nc.gpsimd.dma_gather(
                                out_ap=xlg[:, toff + t0:toff + t0 + ntc, :],
                                in_ap=src_ap,
                                idxs_ap=gidx_sb[:, col0:col0 + ntc * 8],
                                num_idxs=ntc * 128, num_idxs_reg=ntc * 128,
                                elem_size=HC, queue_num=next_q(),
                            )
                    S_tiles = []
                    e_blk = pb.tile([128, T, H], F32, tag="e")
                    for t in range(T):
                        S = pbs.tile([128, 128], BF16, tag="S")
                        S_tiles.append(S)
                        nc.vector.tensor_tensor(
                            out=S[:], in0=dstl_sb[:, b * T + t:b * T + t + 1].to_broadcast([128, 128]),
                            in1=iota[:], op=OP.is_equal)
                        stp = pbp.tile([128, 128], BF16, space="PSUM", tag="stp")
                        nc.tensor.transpose(out=stp[:], in_=S[:], identity=identb[:])
                        ST = pb.tile([128, 128], BF16, tag="ST")
                        nc.scalar.copy(out=ST[:], in_=stp[:])
                        mp = pbp.tile([128, HC], F32, space="PSUM", tag="mp")
                        nc.tensor.matmul(out=mp[:], lhsT=ST[:], rhs=xr_sh[:, b, :], start=True, stop=False)
                        nc.tensor.matmul(out=mp[:], lhsT=identb[:], rhs=xlg[:, t, :], start=False, stop=True)
                        lr = pb.tile([128, HC], BF16, tag="lr")
                        nc.scalar.activation(out=lr[:], in_=mp[:], func=AF.Prelu, alpha=NEG_SLOPE)
                        am = pb.tile([128, HC], BF16, tag="am")
                        nc.vector.tensor_tensor(out=am[:], in0=lr[:], in1=att1b[:], op=OP.mult)
                        nc.vector.tensor_reduce(
                            out=e_blk[:, t, :], in_=am[:].rearrange("p (h c) -> p h c", h=H),
                            axis=mybir.AxisListType.X, op=OP.add)
                    exb = pb.tile([128, T, H], BF16, tag="ex")
                    nc.scalar.activation(
                        out=exb[:].rearrange("p t h -> p (t h)"),
                        in_=e_blk[:].rearrange("p t h -> p (t h)"), func=AF.Exp)
                    agg = pba.tile([128, HC], F32, space="PSUM", tag="agg")
                    den = pba.tile([128, H], F32, space="PSUM", tag="den")
                    for t in range(T):
                        xlx = pb.tile([128, HC], BF16, tag="xlx")
                        nc.vector.tensor_tensor(
                            out=xlx[:].rearrange("p (h c) -> p h c", h=H),
                            in0=xlg[:, t, :].rearrange("p (h c) -> p h c", h=H),
                            in1=exb[:, t, :, None].to_broadcast([128, H, C]),
                            op=OP.mult)
                        nc.tensor.matmul(out=agg[:], lhsT=S_tiles[t][:], rhs=xlx[:],
                                         start=(t == 0), stop=(t == T - 1))
                        nc.tensor.matmul(out=den[:], lhsT=S_tiles[t][:], rhs=exb[:, t, :],
                                         start=(t == 0), stop=(t == T - 1))
                    rd = pb.tile([128, H], F32, tag="rd")
                    nc.vector.reciprocal(out=rd[:], in_=den[:])
                    nc.vector.tensor_tensor(
                        out=h_sh[:, b, :].rearrange("p (h c) -> p h c", h=H),
                        in0=agg[:].rearrange("p (h c) -> p h c", h=H),
                        in1=rd[:, :, None].to_broadcast([128, H, C]), op=OP.mult)

            # ---- bias1 + ELU (batched over groups of blocks) ----
            GE = 8
            with tc.tile_pool(name="pe_sb", bufs=2) as pe:
                for g0 in range(0, NB, GE):
                    ng = min(GE, NB - g0)
                    view = h_sh[:, g0:g0 + ng, :]
                    nc.vector.tensor_tensor(
                        out=view, in0=view,
                        in1=b1r[:, None, :].to_broadcast([128, ng, HC]), op=OP.add)
                    negt = pe.tile([128, GE, HC], F32, tag="neg")
                    nc.vector.tensor_scalar(out=negt[:, :ng, :], in0=view, scalar1=0.0,
                                            scalar2=None, op0=OP.min)
                    expt = pe.tile([128, GE, HC], F32, tag="exp")
                    nc.scalar.activation(
                        out=expt[:, :ng, :].rearrange("p g c -> p (g c)"),
                        in_=negt[:, :ng, :].rearrange("p g c -> p (g c)"), func=AF.Exp)
                    nc.vector.tensor_scalar(out=view, in0=view, scalar1=0.0,
                                            scalar2=None, op0=OP.max)
                    nc.vector.tensor_tensor(out=view, in0=view, in1=expt[:, :ng, :], op=OP.add)
                    nc.vector.tensor_scalar(out=view, in0=view, scalar1=-1.0,
                                            scalar2=None, op0=OP.add)

            # ---- Phase C: layer-2 node transforms ----
            with (
                tc.tile_pool(name="pc_sb", bufs=3) as pc,
                tc.tile_pool(name="pc_ps", bufs=2, space="PSUM") as pcp,
            ):
                for i in range(NB):
                    hT = pc.tile([128, 2, 128], BF16, tag="hT")
                    for half in range(2):
                        tp = pcp.tile([128, 128], F32, space="PSUM", tag="tp")
                        nc.tensor.transpose(
                            out=tp[:], in_=h_sh[:, i, half * 128:(half + 1) * 128],
                            identity=identf[:])
                        nc.scalar.copy(out=hT[:, half, :], in_=tp[:])
                    ps2l = pcp.tile([128, DOUT], F32, space="PSUM", tag="ps2l")
                    ps2r = pcp.tile([128, DOUT], F32, space="PSUM", tag="ps2r")
                    for half in range(2):
                        nc.tensor.matmul(out=ps2l[:], lhsT=hT[:, half, :], rhs=w2lb[:, half, :],
                                         start=(half == 0), stop=(half == 1))
                        nc.tensor.matmul(out=ps2r[:], lhsT=hT[:, half, :], rhs=w2rb[:, half, :],
                                         start=(half == 0), stop=(half == 1))
                    xl2t = pc.tile([128, DOUT], F32, tag="xl2t")
                    nc.vector.tensor_tensor(out=xl2t[:], in0=ps2l[:], in1=bl2[:], op=OP.add)
                    nc.sync.dma_start(out=xl2_ag_in[i * 128:(i + 1) * 128, :], in_=xl2t[:])
                    nc.vector.tensor_tensor(out=xr2_sh[:, i, :], in0=ps2r[:], in1=br2[:], op=OP.add)

            nc.gpsimd.collective_compute(
                "AllGather", OP.bypass, replica_groups=[list(range(NCORES))],
                ins=[xl2_ag_in[:].opt()], outs=[xl2_full[:].opt()],
            )

            # ---- Phase D: layer-2 edge loop (1 head, 64 ch) ----
            with (
                tc.tile_pool(name="pd_sb", bufs=2) as pd,
                tc.tile_pool(name="pd_s", bufs=T + 2) as pds,
                tc.tile_pool(name="pd_ps", bufs=2, space="PSUM") as pdp,
                tc.tile_pool(name="pd_acc", bufs=2, space="PSUM") as pda,
            ):
                for b in range(NB):
                    x2g = pd.tile([128, T, DOUT], F32, tag="x2g")
                    for (toff, nt, base) in ((0, T_LO, 0), (T_LO, T_HI, SPLIT)):
                        src_ap = xl2_full[:] if base == 0 else xl2_full[base:, :]
                        for t0 in range(0, nt, GATHER_TILES_PER_CALL):
                            ntc = min(GATHER_TILES_PER_CALL, nt - t0)
                            col0 = (b * T + toff + t0) * 8
                            nc.gpsimd.dma_gather(
                                out_ap=x2g[:, toff + t0:toff + t0 + ntc, :],
                                in_ap=src_ap,
                                idxs_ap=gidx_sb[:, col0:col0 + ntc * 8],
                                num_idxs=ntc * 128, num_idxs_reg=ntc * 128,
                                elem_size=DOUT, queue_num=next_q(),
                            )
                    S_tiles = []
                    e_blk = pd.tile([128, T], F32, tag="e2")
                    for t in range(T):
                        S = pds.tile([128, 128], F32, tag="S2")
                        S_tiles.append(S)
                        nc.vector.tensor_tensor(
                            out=S[:], in0=dstl_sb[:, b * T + t:b * T + t + 1].to_broadcast([128, 128]),
                            in1=iota[:], op=OP.is_equal)
                        stp = pdp.tile([128, 128], F32, space="PSUM", tag="stp2")
                        nc.tensor.transpose(out=stp[:], in_=S[:], identity=identf[:])
                        ST = pd.tile([128, 128], F32, tag="ST2")
                        nc.scalar.copy(out=ST[:], in_=stp[:])
                        mp = pdp.tile([128, DOUT], F32, space="PSUM", tag="mp2")
                        nc.tensor.matmul(out=mp[:], lhsT=ST[:], rhs=xr2_sh[:, b, :], start=True, stop=False)
                        nc.tensor.matmul(out=mp[:], lhsT=identf[:], rhs=x2g[:, t, :], start=False, stop=True)
                        lr = pd.tile([128, DOUT], F32, tag="lr2")
                        nc.scalar.activation(out=lr[:], in_=mp[:], func=AF.Prelu, alpha=NEG_SLOPE)
                        am = pd.tile([128, DOUT], F32, tag="am2")
                        nc.vector.tensor_tensor(out=am[:], in0=lr[:], in1=att2f[:], op=OP.mult)
                        nc.vector.tensor_reduce(
                            out=e_blk[:, t:t + 1], in_=am[:],
                            axis=mybir.AxisListType.X, op=OP.add)
                    exb = pd.tile([128, T], F32, tag="ex2")
                    nc.scalar.activation(out=exb[:], in_=e_blk[:], func=AF.Exp)
                    agg = pda.tile([128, DOUT], F32, space="PSUM", tag="agg2")
                    den = pda.tile([128, 1], F32, space="PSUM", tag="den2")
                    for t in range(T):
                        xlx = pd.tile([128, DOUT], F32, tag="xlx2")
                        nc.vector.tensor_tensor(
                            out=xlx[:], in0=x2g[:, t, :],
                            in1=exb[:, t:t + 1].to_broadcast([128, DOUT]), op=OP.mult)
                        nc.tensor.matmul(out=agg[:], lhsT=S_tiles[t][:], rhs=xlx[:],
                                         start=(t == 0), stop=(t == T - 1))
                        nc.tensor.matmul(out=den[:], lhsT=S_tiles[t][:], rhs=exb[:, t:t + 1],
                                         start=(t == 0), stop=(t == T - 1))
                    rd = pd.tile([128, 1], F32, tag="rd2")
                    nc.vector.reciprocal(out=rd[:], in_=den[:])
                    nc.vector.tensor_tensor(
                        out=o1_sh[:, b, :], in0=agg[:],
                        in1=rd[:, 0:1].to_broadcast([128, DOUT]), op=OP.mult)

            # ---- Phase E: bias2 + outputs + log_softmax ----
            with tc.tile_pool(name="pf_sb", bufs=1) as pf:
                nc.vector.tensor_tensor(
                    out=o1_sh[:], in0=o1_sh[:],
                    in1=b2r[:, None, :].to_broadcast([128, NB, DOUT]), op=OP.add)
                nc.sync.dma_start(
                    out=out1.rearrange("(b p) c -> p b c", p=128), in_=o1_sh[:])
                rmax = pf.tile([128, NB], F32)
                nc.vector.tensor_reduce(out=rmax[:], in_=o1_sh[:],
                                        axis=mybir.AxisListType.X, op=OP.max)
                xm = pf.tile([128, NB, DOUT], F32)
                nc.vector.tensor_tensor(
                    out=xm[:], in0=o1_sh[:],
                    in1=rmax[:, :, None].to_broadcast([128, NB, DOUT]), op=OP.subtract)
                pexp = pf.tile([128, NB, DOUT], F32)
                nc.scalar.activation(out=pexp[:].rearrange("p b c -> p (b c)"),
                                     in_=xm[:].rearrange("p b c -> p (b c)"), func=AF.Exp)
                ssum = pf.tile([128, NB], F32)
                nc.vector.tensor_reduce(out=ssum[:], in_=pexp[:],
                                        axis=mybir.AxisListType.X, op=OP.add)
                lns = pf.tile([128, NB], F32)
                nc.scalar.activation(out=lns[:], in_=ssum[:], func=AF.Ln)
                nc.vector.tensor_tensor(
                    out=xm[:], in0=xm[:],
                    in1=lns[:, :, None].to_broadcast([128, NB, DOUT]), op=OP.subtract)
                nc.sync.dma_start(
                    out=out2.rearrange("(b p) c -> p b c", p=128), in_=xm[:])

    nc.compile()
    return nc


def kernel(x, edge_index, Wl1, bl1, Wr1, br1, att1, bias1,
           Wl2, bl2, Wr2, br2, att2, bias2):
    x = np.asarray(x, np.float32)
    edge_index = np.asarray(edge_index)
    P = _prep_host(x, edge_index)
    nc = _build_bass(P)

    rep = lambda v, w: np.tile(np.asarray(v, np.float32).reshape(1, -1), (128, 1))[:, :w]
    consts = {
        "iota": np.tile(np.arange(128, dtype=np.float32)[None, :], (128, 1)),
        "identb": np.eye(128).astype(ml_dtypes.bfloat16),
        "identf": np.eye(128, dtype=np.float32),
        "att1r": np.tile(np.asarray(att1, np.float32).reshape(1, HC), (128, 1)),
        "att2r": np.tile(np.asarray(att2, np.float32).reshape(1, DOUT), (128, 1)),
        "w1l": np.asarray(Wl1, np.float32), "w1r": np.asarray(Wr1, np.float32),
        "w2l": np.asarray(Wl2, np.float32), "w2r": np.asarray(Wr2, np.float32),
        "bl1r": rep(bl1, HC), "br1r": rep(br1, HC), "b1r": rep(bias1, HC),
        "bl2r": rep(bl2, DOUT), "br2r": rep(br2, DOUT), "b2r": rep(bias2, DOUT),
    }
    in_maps = []
    for c in range(NCORES):
        m = dict(consts)
        m["xT"] = P["xT_sh"][c]
        m["gidx"] = P["gidx"][c]
        m["dstl"] = P["dstl"][c]
        in_maps.append(m)

    trace = bool(os.environ.get("KERNEL_TRACE"))
    res = bass_utils.run_bass_kernel_spmd(
        nc, in_maps, core_ids=list(range(NCORES)), trace=trace)
    if trace and res.exec_time_ns:
        print(f"HW exec time: {res.exec_time_ns} ns")
        kernel.last_results = res

    counts = P["counts"]
    h = np.concatenate([res.results[c]["out1"][:counts[c]] for c in range(NCORES)], axis=0)
    ls = np.concatenate([res.results[c]["out2"][:counts[c]] for c in range(NCORES)], axis=0)
    return h, ls


# revision 10
# speedup vs baseline: 1.0333x; 1.0333x over previous
"""Trainium2 Bass kernel for 2-layer GATv2 (nn_EvenLamerGAT).

Strategy (8 NeuronCores, SPMD single launch):
  - Host: append self-loops, sort edges by dst, partition dst nodes into 8
    contiguous ranges with ~equal edge counts. Each core owns one dst range.
  - Per core: compute xl/xr node transforms for its node shard (PE matmuls),
    AllGather the xl shards (bf16) into a replicated table, then process its
    edges in dst-blocks of 128 nodes: dma_gather the xl rows per edge,
    build one-hot incidence matrices from the local dst ids, and do the
    segment softmax + aggregation entirely with PE matmuls.
  - Layer 2 repeats the pattern with the layer-1 output (one AllGather of the
    xl2 shards), then log_softmax.

All schedule shapes (tile counts, paddings) are derived from the actual
edge_index passed to kernel(); the Bass program is compiled per call.
"""
import os
import sys

sys.path.insert(0, "/opt/trn_rl_repo")

import numpy as np
import ml_dtypes

from concourse import bass, mybir, bacc, tile
from concourse import bass_utils

F32 = mybir.dt.float32
BF16 = mybir.dt.bfloat16
I16 = mybir.dt.int16
AF = mybir.ActivationFunctionType
OP = mybir.AluOpType

NCORES = 8
SPLIT = 32768          # int16 gather index limit
NEG_SLOPE = 0.2
H, C = 8, 32
HC = H * C             # 256
DIN = 128
DOUT = 64
GATHER_TILES_PER_CALL = 8   # 1024 idxs per dma_gather (hard per-call cap)
NQ = 4                 # SWDGE queues


def _wrap_idx16(idx, num):
    """Wrap `num` int16 indices into the [128, num//16] dma_gather layout."""
    assert num % 128 == 0 and len(idx) == num
    w = np.zeros((128, num // 16), np.int16)
    blk = idx.reshape(num // 16, 16).T
    for g in range(8):
        w[g * 16:(g + 1) * 16, :] = blk
    return w


def _prep_host(x, edge_index):
    N = x.shape[0]
    src = np.concatenate([edge_index[0], np.arange(N, dtype=np.int64)]).astype(np.int64)
    dst = np.concatenate([edge_index[1], np.arange(N, dtype=np.int64)]).astype(np.int64)
    order = np.argsort(dst, kind="stable")
    src_s = src[order].astype(np.int64)
    dst_s = dst[order].astype(np.int64)
    Etot = len(src_s)

    # core ranges: contiguous node spans with ~equal edge counts
    deg = np.bincount(dst_s, minlength=N)
    cum = np.cumsum(deg)
    starts = [0]
    for k in range(1, NCORES):
        starts.append(int(np.searchsorted(cum, k * Etot / NCORES)))
    starts.append(N)
    starts = np.array(starts, np.int64)
    counts = starts[1:] - starts[:-1]
    Np = int(np.ceil(counts.max() / 128) * 128)
    NB = Np // 128
    assert NCORES * Np < 2 * SPLIT, "row index must fit int16 after lo/hi split"

    # global node -> replicated-table row
    owner = np.searchsorted(starts[1:], np.arange(N), side="right")
    table_row = owner * Np + (np.arange(N) - starts[owner])
    src_row = table_row[src_s]
    edge_start = np.searchsorted(dst_s, starts[:-1])
    edge_end = np.searchsorted(dst_s, starts[1:])

    # First pass: per (core, block) lo/hi edge lists (row, dstl)
    per_block = []  # [core][block] = (lo_rows, lo_dstl, hi_rows, hi_dstl)
    T_LO = np.ones(NB, np.int64)
    T_HI = np.ones(NB, np.int64)
    for c in range(NCORES):
        s0, n_c = starts[c], counts[c]
        blocks = []
        e0, e1 = edge_start[c], edge_end[c]
        er = src_row[e0:e1]
        ed = dst_s[e0:e1] - s0            # local dst 0..n_c-1
        for b in range(NB):
            lo_d, hi_d = b * 128, (b + 1) * 128
            m = (ed >= lo_d) & (ed < hi_d)
            rows = er[m]
            dl = (ed[m] - lo_d).astype(np.float32)
            lo = rows < SPLIT
            lo_rows = rows[lo].astype(np.int64)
            lo_dstl = dl[lo]
            hi_rows = rows[~lo] - SPLIT
            hi_dstl = dl[~lo]
            # dummy edges so padded dst slots have nonzero denominators
            nreal = max(0, min(128, n_c - lo_d))
            if nreal < 128:
                pad_d = np.arange(nreal, 128, dtype=np.float32)
                lo_rows = np.concatenate([lo_rows, np.zeros(len(pad_d), np.int64)])
                lo_dstl = np.concatenate([lo_dstl, pad_d])
            blocks.append((lo_rows, lo_dstl, hi_rows, hi_dstl))
            T_LO[b] = max(T_LO[b], (len(lo_rows) + 127) // 128)
            T_HI[b] = max(T_HI[b], (len(hi_rows) + 127) // 128)
        per_block.append(blocks)

    TB = T_LO + T_HI                       # per-block tile count
    dcol = np.zeros(NB + 1, np.int64)      # dstl column offset per block
    dcol[1:] = np.cumsum(TB)
    TCOLS = int(dcol[-1])
    # Second pass: fill padded arrays
    gidx = np.zeros((NCORES, 128, TCOLS * 8), np.int16)
    dstl = np.full((NCORES, 128, TCOLS), 300.0, np.float32)
    for c in range(NCORES):
        for b in range(NB):
            lo_rows, lo_dstl, hi_rows, hi_dstl = per_block[c][b]
            for (rows, dls, toff, nt) in (
                (lo_rows, lo_dstl, 0, int(T_LO[b])),
                (hi_rows, hi_dstl, int(T_LO[b]), int(T_HI[b])),
            ):
                n = nt * 128
                ridx = np.zeros(n, np.int64)
                ridx[: len(rows)] = rows
                dpad = np.full(n, 300.0, np.float32)
                dpad[: len(dls)] = dls
                c0 = int(dcol[b]) + toff
                # dstl layout: slot k=(t*128+p) -> [p, c0+t]
                dstl[c][:, c0:c0 + nt] = dpad.reshape(nt, 128).T
                # gather idx layout: wrapped per call chunk
                for t0 in range(0, nt, GATHER_TILES_PER_CALL):
                    ntc = min(GATHER_TILES_PER_CALL, nt - t0)
                    chunk = ridx[t0 * 128:(t0 + ntc) * 128].astype(np.int16)
                    col0 = (c0 + t0) * 8
                    gidx[c][:, col0:col0 + ntc * 8] = _wrap_idx16(chunk, ntc * 128)

    xT = np.ascontiguousarray(x.T)  # [128, N]
    xT_sh = np.zeros((NCORES, DIN, Np), np.float32)
    for c in range(NCORES):
        xT_sh[c, :, : counts[c]] = xT[:, starts[c]:starts[c] + counts[c]]

    return dict(N=N, starts=starts, counts=counts, Np=Np, NB=NB,
                T_LO=T_LO, T_HI=T_HI, TB=TB, dcol=dcol, TCOLS=TCOLS,
                gidx=gidx, dstl=dstl, xT_sh=xT_sh)


def _build_bass(P):
    """Build the SPMD Bass program for prep dict P."""
    Np, NB = P["Np"], P["NB"]
    T_LO, T_HI, TB, dcol, TCOLS = P["T_LO"], P["T_HI"], P["TB"], P["dcol"], P["TCOLS"]
    TMAX = int(TB.max())
    NROWS = NCORES * Np

    nc = bacc.Bacc("TRN2", target_bir_lowering=False, debug=False,
                   enable_asserts=True, num_devices=NCORES, num_swdge_queues=NQ)

    din = lambda n, s, d: nc.dram_tensor(n, s, d, kind="ExternalInput").ap()
    xT_in = din("xT", [DIN, Np], F32)
    gidx_in = din("gidx", [128, TCOLS * 8], I16)
    dstl_in = din("dstl", [128, TCOLS], F32)
    iota_in = din("iota", [128, 128], F32)
    identb_in = din("identb", [128, 128], BF16)
    identf_in = din("identf", [128, 128], F32)
    att1_in = din("att1r", [128, HC], F32)
    att2_in = din("att2r", [128, DOUT], F32)
    w1l_in = din("w1l", [DIN, HC], F32)
    w1r_in = din("w1r", [DIN, HC], F32)
    w2l_in = din("w2l", [HC, DOUT], F32)
    w2r_in = din("w2r", [HC, DOUT], F32)
    bl1_in = din("bl1r", [128, HC], F32)
    br1_in = din("br1r", [128, HC], F32)
    b1_in = din("b1r", [128, HC], F32)
    bl2_in = din("bl2r", [128, DOUT], F32)
    br2_in = din("br2r", [128, DOUT], F32)
    b2_in = din("b2r", [128, DOUT], F32)
    out1 = nc.dram_tensor("out1", [Np, DOUT], F32, kind="ExternalOutput").ap()
    out2 = nc.dram_tensor("out2", [Np, DOUT], F32, kind="ExternalOutput").ap()

    qctr = [0]

    def next_q():
        q = qctr[0] % NQ
        qctr[0] += 1
        return q

    with tile.TileContext(nc) as tc:
        import contextlib
        with contextlib.ExitStack() as ctx:
            cn = ctx.enter_context(tc.tile_pool(name="const", bufs=1))
            dr = ctx.enter_context(tc.tile_pool(name="dram", bufs=1, space="DRAM"))

            def load_const(ap_in, shape, dt, cast=False):
                t = cn.tile(shape, dt, tag=ap_in.tensor.name)
                (nc.gpsimd if cast else nc.sync).dma_start(out=t[:], in_=ap_in[:])
                return t

            iota = load_const(iota_in, [128, 128], F32)
            identb = load_const(identb_in, [128, 128], BF16)
            identf = load_const(identf_in, [128, 128], F32)
            att1b = load_const(att1_in, [128, HC], BF16, cast=True)
            att2f = load_const(att2_in, [128, DOUT], F32)
            w1l = load_const(w1l_in, [DIN, HC], F32)
            w1r = load_const(w1r_in, [DIN, HC], F32)
            bl1 = load_const(bl1_in, [128, HC], F32)
            br1 = load_const(br1_in, [128, HC], F32)
            b1r = load_const(b1_in, [128, HC], F32)
            bl2 = load_const(bl2_in, [128, DOUT], F32)
            br2 = load_const(br2_in, [128, DOUT], F32)
            b2r = load_const(b2_in, [128, DOUT], F32)
            # W2 as [128, 2, DOUT] bf16 (rows 0:128, 128:256)
            w2lb = cn.tile([128, 2, DOUT], BF16)
            nc.gpsimd.dma_start(out=w2lb[:, 0, :], in_=w2l_in[0:128, :])
            nc.gpsimd.dma_start(out=w2lb[:, 1, :], in_=w2l_in[128:256, :])
            w2rb = cn.tile([128, 2, DOUT], BF16)
            nc.gpsimd.dma_start(out=w2rb[:, 0, :], in_=w2r_in[0:128, :])
            nc.gpsimd.dma_start(out=w2rb[:, 1, :], in_=w2r_in[128:256, :])
            gidx_sb = cn.tile([128, TCOLS * 8], I16)
            nc.sync.dma_start(out=gidx_sb[:], in_=gidx_in[:])
            dstl_sb = cn.tile([128, TCOLS], F32)
            nc.sync.dma_start(out=dstl_sb[:], in_=dstl_in[:])

            # DRAM buffers
            xl_ag_in = dr.tile([Np, HC], BF16)
            xl_full = dr.tile([NROWS, HC], BF16, addr_space="Shared")
            xl2_ag_in = dr.tile([Np, DOUT], F32)
            xl2_full = dr.tile([NROWS, DOUT], F32, addr_space="Shared")

            persist1 = ctx.enter_context(tc.tile_pool(name="persist1", bufs=1))
            xr_sh = persist1.tile([128, NB, HC], BF16)
            h_sh = persist1.tile([128, NB, HC], F32)
            persist2 = ctx.enter_context(tc.tile_pool(name="persist2", bufs=1))
            xr2_sh = persist2.tile([128, NB, DOUT], F32)
            o1_sh = persist2.tile([128, NB, DOUT], F32)

            # ---- Phase A: xl/xr shard transforms ----
            with (
                tc.tile_pool(name="pa_sb", bufs=3) as pa,
                tc.tile_pool(name="pa_ps", bufs=2, space="PSUM") as pap,
            ):
                for i in range(NB):
                    xt = pa.tile([128, 128], F32, tag="xt")
                    nc.sync.dma_start(out=xt[:], in_=xT_in[:, i * 128:(i + 1) * 128])
                    psl = pap.tile([128, HC], F32, space="PSUM", tag="psl")
                    nc.tensor.matmul(out=psl[:], lhsT=xt[:], rhs=w1l[:], start=True, stop=True)
                    xlt = pa.tile([128, HC], BF16, tag="xlt")
                    nc.vector.tensor_tensor(out=xlt[:], in0=psl[:], in1=bl1[:], op=OP.add)
                    nc.sync.dma_start(out=xl_ag_in[i * 128:(i + 1) * 128, :], in_=xlt[:])
                    psr = pap.tile([128, HC], F32, space="PSUM", tag="psr")
                    nc.tensor.matmul(out=psr[:], lhsT=xt[:], rhs=w1r[:], start=True, stop=True)
                    nc.vector.tensor_tensor(out=xr_sh[:, i, :], in0=psr[:], in1=br1[:], op=OP.add)

            nc.gpsimd.collective_compute(
                "AllGather", OP.bypass, replica_groups=[list(range(NCORES))],
                ins=[xl_ag_in[:].opt()], outs=[xl_full[:].opt()],
            )

            # ---- Phase B: layer-1 edge loop (grouped ops, per-block tile counts) ----
            with (
                tc.tile_pool(name="pb_sb", bufs=2) as pb,
                tc.tile_pool(name="pb_s", bufs=(TMAX + 3) // 4 + 1) as pbs,
                tc.tile_pool(name="pb_stp", bufs=1, space="PSUM") as pbstp,
                tc.tile_pool(name="pb_mp", bufs=2, space="PSUM") as pbmp,
                tc.tile_pool(name="pb_agg", bufs=2, space="PSUM") as pbagg,
                tc.tile_pool(name="pb_den", bufs=1, space="PSUM") as pbden,
            ):
                stcnt = [0]
                for b in range(NB):
                    tlo, thi, tb = int(T_LO[b]), int(T_HI[b]), int(TB[b])
                    c0 = int(dcol[b])
                    xlg = pb.tile([128, TMAX, HC], BF16, tag="xlg")
                    for (toff, nt, base) in ((0, tlo, 0), (tlo, thi, SPLIT)):
                        src_ap = xl_full[:] if base == 0 else xl_full[base:, :]
                        for t0 in range(0, nt, GATHER_TILES_PER_CALL):
                            ntc = min(GATHER_TILES_PER_CALL, nt - t0)
                            col0 = (c0 + toff + t0) * 8
                            nc.gpsimd.dma_gather(
                                out_ap=xlg[:, toff + t0:toff + t0 + ntc, :],
                                in_ap=src_ap,
                                idxs_ap=gidx_sb[:, col0:col0 + ntc * 8],
                                num_idxs=ntc * 128, num_idxs_reg=ntc * 128,
                                elem_size=HC, queue_num=next_q(),
                            )
                    S_groups = []
                    ST_tiles = []
                    e_blk = pb.tile([128, TMAX, H], F32, tag="e")
                    lr_blk = pb.tile([128, TMAX, HC], BF16, tag="lr")
                    # pass 1: S, ST, m, prelu, e  (groups of 4; psum m pairs)
                    for g0 in range(0, tb, 4):
                        gs = min(4, tb - g0)
                        S4 = pbs.tile([128, 4, 128], BF16, tag="S4")
                        S_groups.append(S4)
                        nc.vector.tensor_tensor(
                            out=S4[:, :gs, :],
                            in0=dstl_sb[:, c0 + g0:c0 + g0 + gs, None].to_broadcast([128, gs, 128]),
                            in1=iota[:, None, :].to_broadcast([128, gs, 128]),
                            op=OP.is_equal)
                        for j in range(gs):
                            stp = pbstp.tile([128, 128], BF16, space="PSUM", tag="stp")
                            nc.tensor.transpose(out=stp[:], in_=S4[:, j, :], identity=identb[:])
                            ST = pb.tile([128, 128], BF16, tag=f"ST{j % 2}")
                            ST_tiles.append(ST)
                            if stcnt[0] % 2 == 0:
                                nc.scalar.copy(out=ST[:], in_=stp[:])
                            else:
                                nc.vector.tensor_copy(out=ST[:], in_=stp[:])
                            stcnt[0] += 1
                        for j0 in range(0, gs, 2):
                            js = min(2, gs - j0)
                            mp = pbmp.tile([128, 2, 512], F32, space="PSUM", tag="mp")
                            for j in range(js):
                                nc.tensor.matmul(out=mp[:, j, 0:HC], lhsT=ST_tiles[g0 + j0 + j][:],
                                                 rhs=xr_sh[:, b, :], start=True, stop=False)
                            for j in range(js):
                                nc.tensor.matmul(out=mp[:, j, 0:HC], lhsT=identb[:],
                                                 rhs=xlg[:, g0 + j0 + j, :], start=False, stop=True)
                            nc.scalar.activation(
                                out=lr_blk[:, g0 + j0:g0 + j0 + js, :],
                                in_=mp[:, :js, 0:HC], func=AF.Prelu, alpha=NEG_SLOPE)
                        am4 = pb.tile([128, 4, HC], BF16, tag="am4")
                        nc.vector.tensor_tensor(
                            out=am4[:, :gs, :], in0=lr_blk[:, g0:g0 + gs, :],
                            in1=att1b[:, None, :].to_broadcast([128, gs, HC]), op=OP.mult)
                        nc.vector.tensor_reduce(
                            out=e_blk[:, g0:g0 + gs, :],
                            in_=am4[:, :gs, :].rearrange("p g (h c) -> p g h c", h=H),
                            axis=mybir.AxisListType.X, op=OP.add)
                    exb = pb.tile([128, TMAX, H], BF16, tag="ex")
                    nc.scalar.activation(
                        out=exb[:, :tb, :].rearrange("p t h -> p (t h)"),
                        in_=e_blk[:, :tb, :].rearrange("p t h -> p (t h)"), func=AF.Exp)
                    # pass 2: xlx, agg, den
                    agg = pbagg.tile([128, HC], F32, space="PSUM", tag="agg")
                    den = pbden.tile([128, H], F32, space="PSUM", tag="den")
                    for g0 in range(0, tb, 4):
                        gs = min(4, tb - g0)
                        S4 = S_groups[g0 // 4]
                        xlx4 = pb.tile([128, 4, HC], BF16, tag="xlx4")
                        nc.vector.tensor_tensor(
                            out=xlx4[:, :gs, :].rearrange("p g (h c) -> p g h c", h=H),
                            in0=xlg[:, g0:g0 + gs, :].rearrange("p g (h c) -> p g h c", h=H),
                            in1=exb[:, g0:g0 + gs, :, None].to_broadcast([128, gs, H, C]),
                            op=OP.mult)
                        for j in range(gs):
                            t = g0 + j
                            nc.tensor.matmul(out=agg[:], lhsT=S4[:, j, :], rhs=xlx4[:, j, :],
                                             start=(t == 0), stop=(t == tb - 1))
                            nc.tensor.matmul(out=den[:], lhsT=S4[:, j, :], rhs=exb[:, t, :],
                                             start=(t == 0), stop=(t == tb - 1))
                    rd = pb.tile([128, H], F32, tag="rd")
                    nc.vector.reciprocal(out=rd[:], in_=den[:])
                    nc.vector.tensor_tensor(
                        out=h_sh[:, b, :].rearrange("p (h c) -> p h c", h=H),
                        in0=agg[:].rearrange("p (h c) -> p h c", h=H),
                        in1=rd[:, :, None].to_broadcast([128, H, C]), op=OP.mult)

            # ---- bias1 + ELU (batched over groups of blocks) ----
            GE = 8
            with tc.tile_pool(name="pe_sb", bufs=2) as pe:
                for g0 in range(0, NB, GE):
                    ng = min(GE, NB - g0)
                    view = h_sh[:, g0:g0 + ng, :]
                    nc.vector.tensor_tensor(
                        out=view, in0=view,
                        in1=b1r[:, None, :].to_broadcast([128, ng, HC]), op=OP.add)
                    negt = pe.tile([128, GE, HC], F32, tag="neg")
                    nc.vector.tensor_scalar(out=negt[:, :ng, :], in0=view, scalar1=0.0,
                                            scalar2=None, op0=OP.min)
                    expt = pe.tile([128, GE, HC], F32, tag="exp")
                    nc.scalar.activation(
                        out=expt[:, :ng, :].rearrange("p g c -> p (g c)"),
                        in_=negt[:, :ng, :].rearrange("p g c -> p (g c)"), func=AF.Exp)
                    nc.vector.tensor_scalar(out=view, in0=view, scalar1=0.0,
                                            scalar2=None, op0=OP.max)
                    nc.vector.tensor_tensor(out=view, in0=view, in1=expt[:, :ng, :], op=OP.add)
                    nc.vector.tensor_scalar(out=view, in0=view, scalar1=-1.0,
                                            scalar2=None, op0=OP.add)

            # ---- Phase C: layer-2 node transforms ----
            with (
                tc.tile_pool(name="pc_sb", bufs=3) as pc,
                tc.tile_pool(name="pc_ps", bufs=2, space="PSUM") as pcp,
            ):
                for i in range(NB):
                    hT = pc.tile([128, 2, 128], BF16, tag="hT")
                    for half in range(2):
                        tp = pcp.tile([128, 128], F32, space="PSUM", tag="tp")
                        nc.tensor.transpose(
                            out=tp[:], in_=h_sh[:, i, half * 128:(half + 1) * 128],
                            identity=identf[:])
                        nc.scalar.copy(out=hT[:, half, :], in_=tp[:])
                    ps2l = pcp.tile([128, DOUT], F32, space="PSUM", tag="ps2l")
                    ps2r = pcp.tile([128, DOUT], F32, space="PSUM", tag="ps2r")
                    for half in range(2):
                        nc.tensor.matmul(out=ps2l[:], lhsT=hT[:, half, :], rhs=w2lb[:, half, :],
                                         start=(half == 0), stop=(half == 1))
                        nc.tensor.matmul(out=ps2r[:], lhsT=hT[:, half, :], rhs=w2rb[:, half, :],
                                         start=(half == 0), stop=(half == 1))
                    xl2t = pc.tile([128, DOUT], F32, tag="xl2t")
                    nc.vector.tensor_tensor(out=xl2t[:], in0=ps2l[:], in1=bl2[:], op=OP.add)
                    nc.sync.dma_start(out=xl2_ag_in[i * 128:(i + 1) * 128, :], in_=xl2t[:])
                    nc.vector.tensor_tensor(out=xr2_sh[:, i, :], in0=ps2r[:], in1=br2[:], op=OP.add)

            nc.gpsimd.collective_compute(
                "AllGather", OP.bypass, replica_groups=[list(range(NCORES))],
                ins=[xl2_ag_in[:].opt()], outs=[xl2_full[:].opt()],
            )

            # ---- Phase D: layer-2 edge loop (1 head, 64 ch, grouped) ----
            with (
                tc.tile_pool(name="pd_sb", bufs=2) as pd,
                tc.tile_pool(name="pd_s", bufs=(TMAX + 3) // 4 + 1) as pds,
                tc.tile_pool(name="pd_stp", bufs=1, space="PSUM") as pdstp,
                tc.tile_pool(name="pd_mp", bufs=2, space="PSUM") as pdmp,
                tc.tile_pool(name="pd_agg", bufs=2, space="PSUM") as pdagg,
                tc.tile_pool(name="pd_den", bufs=1, space="PSUM") as pdden,
            ):
                stcnt = [0]
                for b in range(NB):
                    tlo, thi, tb = int(T_LO[b]), int(T_HI[b]), int(TB[b])
                    c0 = int(dcol[b])
                    x2g = pd.tile([128, TMAX, DOUT], F32, tag="x2g")
                    for (toff, nt, base) in ((0, tlo, 0), (tlo, thi, SPLIT)):
                        src_ap = xl2_full[:] if base == 0 else xl2_full[base:, :]
                        for t0 in range(0, nt, GATHER_TILES_PER_CALL):
                            ntc = min(GATHER_TILES_PER_CALL, nt - t0)
                            col0 = (c0 + toff + t0) * 8
                            nc.gpsimd.dma_gather(
                                out_ap=x2g[:, toff + t0:toff + t0 + ntc, :],
                                in_ap=src_ap,
                                idxs_ap=gidx_sb[:, col0:col0 + ntc * 8],
                                num_idxs=ntc * 128, num_idxs_reg=ntc * 128,
                                elem_size=DOUT, queue_num=next_q(),
                            )
                    S_groups = []
                    ST_tiles = []
                    e_blk = pd.tile([128, TMAX], F32, tag="e2")
                    lr_blk = pd.tile([128, TMAX, DOUT], F32, tag="lr2")
                    for g0 in range(0, tb, 4):
                        gs = min(4, tb - g0)
                        S4 = pds.tile([128, 4, 128], F32, tag="S24")
                        S_groups.append(S4)
                        nc.vector.tensor_tensor(
                            out=S4[:, :gs, :],
                            in0=dstl_sb[:, c0 + g0:c0 + g0 + gs, None].to_broadcast([128, gs, 128]),
                            in1=iota[:, None, :].to_broadcast([128, gs, 128]),
                            op=OP.is_equal)
                        for j in range(gs):
                            stp = pdstp.tile([128, 128], F32, space="PSUM", tag="stp2")
                            nc.tensor.transpose(out=stp[:], in_=S4[:, j, :], identity=identf[:])
                            ST = pd.tile([128, 128], F32, tag=f"ST2{j % 2}")
                            ST_tiles.append(ST)
                            if stcnt[0] % 2 == 0:
                                nc.scalar.copy(out=ST[:], in_=stp[:])
                            else:
                                nc.vector.tensor_copy(out=ST[:], in_=stp[:])
                            stcnt[0] += 1
                        for j0 in range(0, gs, 2):
                            js = min(2, gs - j0)
                            mp = pdmp.tile([128, 2, 512], F32, space="PSUM", tag="mp2")
                            for j in range(js):
                                nc.tensor.matmul(out=mp[:, j, 0:DOUT], lhsT=ST_tiles[g0 + j0 + j][:],
                                                 rhs=xr2_sh[:, b, :], start=True, stop=False)
                            for j in range(js):
                                nc.tensor.matmul(out=mp[:, j, 0:DOUT], lhsT=identf[:],
                                                 rhs=x2g[:, g0 + j0 + j, :], start=False, stop=True)
                            nc.scalar.activation(
                                out=lr_blk[:, g0 + j0:g0 + j0 + js, :],
                                in_=mp[:, :js, 0:DOUT], func=AF.Prelu, alpha=NEG_SLOPE)
                        am4 = pd.tile([128, 4, DOUT], F32, tag="am24")
                        nc.vector.tensor_tensor(
                            out=am4[:, :gs, :], in0=lr_blk[:, g0:g0 + gs, :],
                            in1=att2f[:, None, :].to_broadcast([128, gs, DOUT]), op=OP.mult)
                        nc.vector.tensor_reduce(
                            out=e_blk[:, g0:g0 + gs], in_=am4[:, :gs, :],
                            axis=mybir.AxisListType.X, op=OP.add)
                    exb = pd.tile([128, TMAX], F32, tag="ex2")
                    nc.scalar.activation(out=exb[:, :tb], in_=e_blk[:, :tb], func=AF.Exp)
                    agg = pdagg.tile([128, DOUT], F32, space="PSUM", tag="agg2")
                    den = pdden.tile([128, 1], F32, space="PSUM", tag="den2")
                    for g0 in range(0, tb, 4):
                        gs = min(4, tb - g0)
                        S4 = S_groups[g0 // 4]
                        xlx4 = pd.tile([128, 4, DOUT], F32, tag="xlx24")
                        nc.vector.tensor_tensor(
                            out=xlx4[:, :gs, :], in0=x2g[:, g0:g0 + gs, :],
                            in1=exb[:, g0:g0 + gs, None].to_broadcast([128, gs, DOUT]),
                            op=OP.mult)
                        for j in range(gs):
                            t = g0 + j
                            nc.tensor.matmul(out=agg[:], lhsT=S4[:, j, :], rhs=xlx4[:, j, :],
                                             start=(t == 0), stop=(t == tb - 1))
                            nc.tensor.matmul(out=den[:], lhsT=S4[:, j, :], rhs=exb[:, t:t + 1],
                                             start=(t == 0), stop=(t == tb - 1))
                    rd = pd.tile([128, 1], F32, tag="rd2")
                    nc.vector.reciprocal(out=rd[:], in_=den[:])
                    nc.vector.tensor_tensor(
                        out=o1_sh[:, b, :], in0=agg[:],
                        in1=rd[:, 0:1].to_broadcast([128, DOUT]), op=OP.mult)

            # ---- Phase E: bias2 + outputs + log_softmax ----
            with tc.tile_pool(name="pf_sb", bufs=1) as pf:
                nc.vector.tensor_tensor(
                    out=o1_sh[:], in0=o1_sh[:],
                    in1=b2r[:, None, :].to_broadcast([128, NB, DOUT]), op=OP.add)
                nc.sync.dma_start(
                    out=out1.rearrange("(b p) c -> p b c", p=128), in_=o1_sh[:])
                rmax = pf.tile([128, NB], F32)
                nc.vector.tensor_reduce(out=rmax[:], in_=o1_sh[:],
                                        axis=mybir.AxisListType.X, op=OP.max)
                xm = pf.tile([128, NB, DOUT], F32)
                nc.vector.tensor_tensor(
                    out=xm[:], in0=o1_sh[:],
                    in1=rmax[:, :, None].to_broadcast([128, NB, DOUT]), op=OP.subtract)
                pexp = pf.tile([128, NB, DOUT], F32)
                nc.scalar.activation(out=pexp[:].rearrange("p b c -> p (b c)"),
                                     in_=xm[:].rearrange("p b c -> p (b c)"), func=AF.Exp)
                ssum = pf.tile([128, NB], F32)
                nc.vector.tensor_reduce(out=ssum[:], in_=pexp[:],
                                        axis=mybir.AxisListType.X, op=OP.add)
                lns = pf.tile([128, NB], F32)
                nc.scalar.activation(out=lns[:], in_=ssum[:], func=AF.Ln)
                nc.vector.tensor_tensor(
                    out=xm[:], in0=xm[:],
                    in1=lns[:, :, None].to_broadcast([128, NB, DOUT]), op=OP.subtract)
                nc.sync.dma_start(
                    out=out2.rearrange("(b p) c -> p b c", p=128), in_=xm[:])

    nc.compile()
    return nc


def kernel(x, edge_index, Wl1, bl1, Wr1, br1, att1, bias1,
           Wl2, bl2, Wr2, br2, att2, bias2):
    x = np.asarray(x, np.float32)
    edge_index = np.asarray(edge_index)
    P = _prep_host(x, edge_index)
    nc = _build_bass(P)

    rep = lambda v, w: np.tile(np.asarray(v, np.float32).reshape(1, -1), (128, 1))[:, :w]
    consts = {
        "iota": np.tile(np.arange(128, dtype=np.float32)[None, :], (128, 1)),
        "identb": np.eye(128).astype(ml_dtypes.bfloat16),
        "identf": np.eye(128, dtype=np.float32),
        "att1r": np.tile(np.asarray(att1, np.float32).reshape(1, HC), (128, 1)),
        "att2r": np.tile(np.asarray(att2, np.float32).reshape(1, DOUT), (128, 1)),
        "w1l": np.asarray(Wl1, np.float32), "w1r": np.asarray(Wr1, np.float32),
        "w2l": np.asarray(Wl2, np.float32), "w2r": np.asarray(Wr2, np.float32),
        "bl1r": rep(bl1, HC), "br1r": rep(br1, HC), "b1r": rep(bias1, HC),
        "bl2r": rep(bl2, DOUT), "br2r": rep(br2, DOUT), "b2r": rep(bias2, DOUT),
    }
    in_maps = []
    for c in range(NCORES):
        m = dict(consts)
        m["xT"] = P["xT_sh"][c]
        m["gidx"] = P["gidx"][c]
        m["dstl"] = P["dstl"][c]
        in_maps.append(m)

    trace = bool(os.environ.get("KERNEL_TRACE"))
    res = bass_utils.run_bass_kernel_spmd(
        nc, in_maps, core_ids=list(range(NCORES)), trace=trace)
    if trace and res.exec_time_ns:
        print(f"HW exec time: {res.exec_time_ns} ns")
        kernel.last_results = res

    counts = P["counts"]
    h = np.concatenate([res.results[c]["out1"][:counts[c]] for c in range(NCORES)], axis=0)
    ls = np.concatenate([res.results[c]["out2"][:counts[c]] for c in range(NCORES)], axis=0)
    return h, ls


# revision 14
# speedup vs baseline: 1.5748x; 1.5241x over previous
"""Trainium2 Bass kernel for 2-layer GATv2 (nn_EvenLamerGAT).

Strategy (8 NeuronCores, SPMD single launch):
  - Host: append self-loops, sort edges by dst, partition dst nodes into 8
    contiguous ranges with ~equal edge counts. Each core owns one dst range.
  - Per core: compute xl/xr node transforms for its node shard (PE matmuls),
    AllGather the xl shards (bf16) into a replicated table, then process its
    edges in dst-blocks of 128 nodes: dma_gather the xl rows per edge,
    build one-hot incidence matrices from the local dst ids, and do the
    segment softmax + aggregation entirely with PE matmuls.
  - Layer 2 repeats the pattern with the layer-1 output (one AllGather of the
    xl2 shards), then log_softmax.

All schedule shapes (tile counts, paddings) are derived from the actual
edge_index passed to kernel(); the Bass program is compiled per call.
"""
import os
import sys

sys.path.insert(0, "/opt/trn_rl_repo")

import numpy as np
import ml_dtypes

from concourse import bass, mybir, bacc, tile
from concourse import bass_utils

F32 = mybir.dt.float32
BF16 = mybir.dt.bfloat16
I16 = mybir.dt.int16
AF = mybir.ActivationFunctionType
OP = mybir.AluOpType

NCORES = 8
SPLIT = 32768          # int16 gather index limit
NEG_SLOPE = 0.2
H, C = 8, 32
HC = H * C             # 256
DIN = 128
DOUT = 64
GATHER_TILES_PER_CALL = 8   # 1024 idxs per dma_gather (hard per-call cap)
NQ = 4                 # SWDGE queues


def _wrap_idx16(idx, num):
    """Wrap `num` int16 indices into the [128, num//16] dma_gather layout."""
    assert num % 128 == 0 and len(idx) == num
    w = np.zeros((128, num // 16), np.int16)
    blk = idx.reshape(num // 16, 16).T
    for g in range(8):
        w[g * 16:(g + 1) * 16, :] = blk
    return w


def _prep_host(x, edge_index):
    N = x.shape[0]
    src = np.concatenate([edge_index[0], np.arange(N, dtype=np.int64)]).astype(np.int64)
    dst = np.concatenate([edge_index[1], np.arange(N, dtype=np.int64)]).astype(np.int64)
    order = np.argsort(dst, kind="stable")
    src_s = src[order].astype(np.int64)
    dst_s = dst[order].astype(np.int64)
    Etot = len(src_s)

    # core ranges: contiguous node spans with ~equal edge counts
    deg = np.bincount(dst_s, minlength=N)
    cum = np.cumsum(deg)
    starts = [0]
    for k in range(1, NCORES):
        starts.append(int(np.searchsorted(cum, k * Etot / NCORES)))
    starts.append(N)
    starts = np.array(starts, np.int64)
    counts = starts[1:] - starts[:-1]
    Np = int(np.ceil(counts.max() / 128) * 128)
    NB = Np // 128
    assert NCORES * Np < 2 * SPLIT, "row index must fit int16 after lo/hi split"

    # global node -> replicated-table row
    owner = np.searchsorted(starts[1:], np.arange(N), side="right")
    table_row = owner * Np + (np.arange(N) - starts[owner])
    src_row = table_row[src_s]
    edge_start = np.searchsorted(dst_s, starts[:-1])
    edge_end = np.searchsorted(dst_s, starts[1:])

    # First pass: per (core, block) lo/hi edge lists (row, dstl)
    per_block = []  # [core][block] = (lo_rows, lo_dstl, hi_rows, hi_dstl)
    T_LO = np.ones(NB, np.int64)
    T_HI = np.ones(NB, np.int64)
    for c in range(NCORES):
        s0, n_c = starts[c], counts[c]
        blocks = []
        e0, e1 = edge_start[c], edge_end[c]
        er = src_row[e0:e1]
        ed = dst_s[e0:e1] - s0            # local dst 0..n_c-1
        for b in range(NB):
            lo_d, hi_d = b * 128, (b + 1) * 128
            m = (ed >= lo_d) & (ed < hi_d)
            rows = er[m]
            dl = (ed[m] - lo_d).astype(np.float32)
            lo = rows < SPLIT
            lo_rows = rows[lo].astype(np.int64)
            lo_dstl = dl[lo]
            hi_rows = rows[~lo] - SPLIT
            hi_dstl = dl[~lo]
            # dummy edges so padded dst slots have nonzero denominators
            nreal = max(0, min(128, n_c - lo_d))
            if nreal < 128:
                pad_d = np.arange(nreal, 128, dtype=np.float32)
                lo_rows = np.concatenate([lo_rows, np.zeros(len(pad_d), np.int64)])
                lo_dstl = np.concatenate([lo_dstl, pad_d])
            blocks.append((lo_rows, lo_dstl, hi_rows, hi_dstl))
            T_LO[b] = max(T_LO[b], (len(lo_rows) + 127) // 128)
            T_HI[b] = max(T_HI[b], (len(hi_rows) + 127) // 128)
        per_block.append(blocks)

    TB = T_LO + T_HI                       # per-block tile count
    dcol = np.zeros(NB + 1, np.int64)      # dstl column offset per block
    dcol[1:] = np.cumsum(TB)
    TCOLS = int(dcol[-1])
    # Second pass: fill padded arrays
    gidx = np.zeros((NCORES, 128, TCOLS * 8), np.int16)
    dstl = np.full((NCORES, 128, TCOLS), 300.0, np.float32)
    for c in range(NCORES):
        for b in range(NB):
            lo_rows, lo_dstl, hi_rows, hi_dstl = per_block[c][b]
            for (rows, dls, toff, nt) in (
                (lo_rows, lo_dstl, 0, int(T_LO[b])),
                (hi_rows, hi_dstl, int(T_LO[b]), int(T_HI[b])),
            ):
                n = nt * 128
                ridx = np.zeros(n, np.int64)
                ridx[: len(rows)] = rows
                dpad = np.full(n, 300.0, np.float32)
                dpad[: len(dls)] = dls
                c0 = int(dcol[b]) + toff
                # dstl layout: slot k=(t*128+p) -> [p, c0+t]
                dstl[c][:, c0:c0 + nt] = dpad.reshape(nt, 128).T
                # gather idx layout: wrapped per call chunk
                for t0 in range(0, nt, GATHER_TILES_PER_CALL):
                    ntc = min(GATHER_TILES_PER_CALL, nt - t0)
                    chunk = ridx[t0 * 128:(t0 + ntc) * 128].astype(np.int16)
                    col0 = (c0 + t0) * 8
                    gidx[c][:, col0:col0 + ntc * 8] = _wrap_idx16(chunk, ntc * 128)

    xT = np.ascontiguousarray(x.T)  # [128, N]
    xT_sh = np.zeros((NCORES, DIN, Np), np.float32)
    for c in range(NCORES):
        xT_sh[c, :, : counts[c]] = xT[:, starts[c]:starts[c] + counts[c]]

    return dict(N=N, starts=starts, counts=counts, Np=Np, NB=NB,
                T_LO=T_LO, T_HI=T_HI, TB=TB, dcol=dcol, TCOLS=TCOLS,
                gidx=gidx, dstl=dstl, xT_sh=xT_sh)


def _build_bass(P):
    """Build the SPMD Bass program for prep dict P."""
    Np, NB = P["Np"], P["NB"]
    T_LO, T_HI, TB, dcol, TCOLS = P["T_LO"], P["T_HI"], P["TB"], P["dcol"], P["TCOLS"]
    TMAX = int(TB.max())
    NROWS = NCORES * Np

    nc = bacc.Bacc("TRN2", target_bir_lowering=False, debug=False,
                   enable_asserts=True, num_devices=NCORES, num_swdge_queues=NQ)

    din = lambda n, s, d: nc.dram_tensor(n, s, d, kind="ExternalInput").ap()
    xT_in = din("xT", [DIN, Np], F32)
    gidx_in = din("gidx", [128, TCOLS * 8], I16)
    dstl_in = din("dstl", [128, TCOLS], F32)
    iota_in = din("iota", [128, 128], F32)
    identb_in = din("identb", [128, 128], BF16)
    identf_in = din("identf", [128, 128], F32)
    att1_in = din("att1r", [128, HC], F32)
    att2_in = din("att2r", [128, DOUT], F32)
    w1l_in = din("w1l", [DIN, HC], F32)
    w1r_in = din("w1r", [DIN, HC], F32)
    w2l_in = din("w2l", [HC, DOUT], F32)
    w2r_in = din("w2r", [HC, DOUT], F32)
    bl1_in = din("bl1r", [128, HC], F32)
    br1_in = din("br1r", [128, HC], F32)
    b1_in = din("b1r", [128, HC], F32)
    bl2_in = din("bl2r", [128, DOUT], F32)
    br2_in = din("br2r", [128, DOUT], F32)
    b2_in = din("b2r", [128, DOUT], F32)
    out1 = nc.dram_tensor("out1", [Np, DOUT], F32, kind="ExternalOutput").ap()
    out2 = nc.dram_tensor("out2", [Np, DOUT], F32, kind="ExternalOutput").ap()

    qctr = [0]

    def next_q():
        q = qctr[0] % NQ
        qctr[0] += 1
        return q

    with tile.TileContext(nc) as tc:
        import contextlib
        with contextlib.ExitStack() as ctx:
            cn = ctx.enter_context(tc.tile_pool(name="const", bufs=1))
            dr = ctx.enter_context(tc.tile_pool(name="dram", bufs=1, space="DRAM"))

            def load_const(ap_in, shape, dt, cast=False):
                t = cn.tile(shape, dt, tag=ap_in.tensor.name)
                (nc.gpsimd if cast else nc.sync).dma_start(out=t[:], in_=ap_in[:])
                return t

            iota = load_const(iota_in, [128, 128], F32)
            identb = load_const(identb_in, [128, 128], BF16)
            identf = load_const(identf_in, [128, 128], F32)
            att1b = load_const(att1_in, [128, HC], BF16, cast=True)
            att2f = load_const(att2_in, [128, DOUT], BF16, cast=True)
            w1l = load_const(w1l_in, [DIN, HC], BF16, cast=True)
            w1r = load_const(w1r_in, [DIN, HC], BF16, cast=True)
            bl1 = load_const(bl1_in, [128, HC], F32)
            br1 = load_const(br1_in, [128, HC], F32)
            b1r = load_const(b1_in, [128, HC], F32)
            bl2 = load_const(bl2_in, [128, DOUT], F32)
            br2 = load_const(br2_in, [128, DOUT], F32)
            b2r = load_const(b2_in, [128, DOUT], F32)
            # W2 as [128, 2, DOUT] bf16 (rows 0:128, 128:256)
            w2lb = cn.tile([128, 2, DOUT], BF16)
            nc.gpsimd.dma_start(out=w2lb[:, 0, :], in_=w2l_in[0:128, :])
            nc.gpsimd.dma_start(out=w2lb[:, 1, :], in_=w2l_in[128:256, :])
            w2rb = cn.tile([128, 2, DOUT], BF16)
            nc.gpsimd.dma_start(out=w2rb[:, 0, :], in_=w2r_in[0:128, :])
            nc.gpsimd.dma_start(out=w2rb[:, 1, :], in_=w2r_in[128:256, :])
            gidx_sb = cn.tile([128, TCOLS * 8], I16)
            nc.sync.dma_start(out=gidx_sb[:], in_=gidx_in[:])
            dstl_sb = cn.tile([128, TCOLS], F32)
            nc.sync.dma_start(out=dstl_sb[:], in_=dstl_in[:])

            # DRAM buffers
            xl_ag_in = dr.tile([Np, HC], BF16)
            xl_full = dr.tile([NROWS, HC], BF16, addr_space="Shared")
            xl2_ag_in = dr.tile([Np, 2 * DOUT], BF16)
            xl2_full = dr.tile([NROWS, 2 * DOUT], BF16, addr_space="Shared")

            persist1 = ctx.enter_context(tc.tile_pool(name="persist1", bufs=1))
            xr_sh = persist1.tile([128, NB, HC], BF16)
            h_sh = persist1.tile([128, NB, HC], BF16)
            persist2 = ctx.enter_context(tc.tile_pool(name="persist2", bufs=1))
            xr2_sh = persist2.tile([128, NB, DOUT], BF16)
            o1_sh = persist2.tile([128, NB, DOUT], F32)

            # ---- Phase A: xl/xr shard transforms ----
            with (
                tc.tile_pool(name="pa_sb", bufs=3) as pa,
                tc.tile_pool(name="pa_ps", bufs=2, space="PSUM") as pap,
            ):
                for i in range(NB):
                    xt = pa.tile([128, 128], BF16, tag="xt")
                    nc.gpsimd.dma_start(out=xt[:], in_=xT_in[:, i * 128:(i + 1) * 128])
                    psl = pap.tile([128, HC], F32, space="PSUM", tag="psl")
                    nc.tensor.matmul(out=psl[:], lhsT=xt[:], rhs=w1l[:], start=True, stop=True)
                    xlt = pa.tile([128, HC], BF16, tag="xlt")
                    nc.vector.tensor_tensor(out=xlt[:], in0=psl[:], in1=bl1[:], op=OP.add)
                    nc.sync.dma_start(out=xl_ag_in[i * 128:(i + 1) * 128, :], in_=xlt[:])
                    psr = pap.tile([128, HC], F32, space="PSUM", tag="psr")
                    nc.tensor.matmul(out=psr[:], lhsT=xt[:], rhs=w1r[:], start=True, stop=True)
                    nc.vector.tensor_tensor(out=xr_sh[:, i, :], in0=psr[:], in1=br1[:], op=OP.add)

            nc.gpsimd.collective_compute(
                "AllGather", OP.bypass, replica_groups=[list(range(NCORES))],
                ins=[xl_ag_in[:].opt()], outs=[xl_full[:].opt()],
            )

            # ---- Phase B: layer-1 edge loop (grouped ops, per-block tile counts) ----
            with (
                tc.tile_pool(name="pb_sb", bufs=2) as pb,
                tc.tile_pool(name="pb_s", bufs=(TMAX + 3) // 4 + 1) as pbs,
                tc.tile_pool(name="pb_stp", bufs=2, space="PSUM") as pbstp,
                tc.tile_pool(name="pb_mp", bufs=2, space="PSUM") as pbmp,
                tc.tile_pool(name="pb_agg", bufs=2, space="PSUM") as pbagg,
            ):
                stcnt = [0]
                for b in range(NB):
                    tlo, thi, tb = int(T_LO[b]), int(T_HI[b]), int(TB[b])
                    c0 = int(dcol[b])
                    xlg = pb.tile([128, TMAX, HC], BF16, tag="xlg")
                    for (toff, nt, base) in ((0, tlo, 0), (tlo, thi, SPLIT)):
                        src_ap = xl_full[:] if base == 0 else xl_full[base:, :]
                        for t0 in range(0, nt, GATHER_TILES_PER_CALL):
                            ntc = min(GATHER_TILES_PER_CALL, nt - t0)
                            col0 = (c0 + toff + t0) * 8
                            nc.gpsimd.dma_gather(
                                out_ap=xlg[:, toff + t0:toff + t0 + ntc, :],
                                in_ap=src_ap,
                                idxs_ap=gidx_sb[:, col0:col0 + ntc * 8],
                                num_idxs=ntc * 128, num_idxs_reg=ntc * 128,
                                elem_size=HC, queue_num=next_q(),
                            )
                    S_groups = []
                    ST_tiles = []
                    e_blk = pb.tile([128, TMAX, H], F32, tag="e")
                    lr_blk = pb.tile([128, TMAX, HC], BF16, tag="lr")
                    # pass 1: S, ST, m, prelu, e  (groups of 4; psum m pairs)
                    for g0 in range(0, tb, 4):
                        gs = min(4, tb - g0)
                        S4 = pbs.tile([128, 4, 128], BF16, tag="S4")
                        S_groups.append(S4)
                        nc.vector.tensor_tensor(
                            out=S4[:, :gs, :],
                            in0=dstl_sb[:, c0 + g0:c0 + g0 + gs, None].to_broadcast([128, gs, 128]),
                            in1=iota[:, None, :].to_broadcast([128, gs, 128]),
                            op=OP.is_equal)
                        for j in range(gs):
                            stp = pbstp.tile([128, 128], BF16, space="PSUM", tag="stp")
                            nc.tensor.transpose(out=stp[:], in_=S4[:, j, :], identity=identb[:])
                            ST = pb.tile([128, 128], BF16, tag=f"ST{j % 2}")
                            ST_tiles.append(ST)
                            if stcnt[0] % 2 == 0:
                                nc.scalar.copy(out=ST[:], in_=stp[:])
                            else:
                                nc.vector.tensor_copy(out=ST[:], in_=stp[:])
                            stcnt[0] += 1
                        for j0 in range(0, gs, 2):
                            js = min(2, gs - j0)
                            mp = pbmp.tile([128, 2, 512], F32, space="PSUM", tag="mp")
                            for j in range(js):
                                nc.tensor.matmul(out=mp[:, j, 0:HC], lhsT=ST_tiles[g0 + j0 + j][:],
                                                 rhs=xr_sh[:, b, :], start=True, stop=False)
                            for j in range(js):
                                nc.tensor.matmul(out=mp[:, j, 0:HC], lhsT=identb[:],
                                                 rhs=xlg[:, g0 + j0 + j, :], start=False, stop=True)
                            nc.scalar.activation(
                                out=lr_blk[:, g0 + j0:g0 + j0 + js, :],
                                in_=mp[:, :js, 0:HC], func=AF.Prelu, alpha=NEG_SLOPE)
                        am4 = pb.tile([128, 4, HC], BF16, tag="am4")
                        nc.vector.tensor_tensor(
                            out=am4[:, :gs, :], in0=lr_blk[:, g0:g0 + gs, :],
                            in1=att1b[:, None, :].to_broadcast([128, gs, HC]), op=OP.mult)
                        nc.vector.tensor_reduce(
                            out=e_blk[:, g0:g0 + gs, :],
                            in_=am4[:, :gs, :].rearrange("p g (h c) -> p g h c", h=H),
                            axis=mybir.AxisListType.X, op=OP.add)
                    exb = pb.tile([128, TMAX, H], BF16, tag="ex")
                    nc.scalar.activation(
                        out=exb[:, :tb, :].rearrange("p t h -> p (t h)"),
                        in_=e_blk[:, :tb, :].rearrange("p t h -> p (t h)"), func=AF.Exp)
                    # pass 2: xlx, agg, den
                    agg = pbagg.tile([128, HC + H], F32, space="PSUM", tag="agg")
                    for g0 in range(0, tb, 4):
                        gs = min(4, tb - g0)
                        S4 = S_groups[g0 // 4]
                        xlx4 = pb.tile([128, 4, HC + H], BF16, tag="xlx4")
                        nc.vector.tensor_tensor(
                            out=xlx4[:, :gs, 0:HC].rearrange("p g (h c) -> p g h c", h=H),
                            in0=xlg[:, g0:g0 + gs, :].rearrange("p g (h c) -> p g h c", h=H),
                            in1=exb[:, g0:g0 + gs, :, None].to_broadcast([128, gs, H, C]),
                            op=OP.mult)
                        nc.vector.tensor_copy(out=xlx4[:, :gs, HC:HC + H],
                                              in_=exb[:, g0:g0 + gs, :])
                        for j in range(gs):
                            t = g0 + j
                            nc.tensor.matmul(out=agg[:], lhsT=S4[:, j, :], rhs=xlx4[:, j, :],
                                             start=(t == 0), stop=(t == tb - 1))
                    rd = pb.tile([128, H], F32, tag="rd")
                    nc.vector.reciprocal(out=rd[:], in_=agg[:, HC:HC + H])
                    nc.vector.tensor_tensor(
                        out=h_sh[:, b, :].rearrange("p (h c) -> p h c", h=H),
                        in0=agg[:, 0:HC].rearrange("p (h c) -> p h c", h=H),
                        in1=rd[:, :, None].to_broadcast([128, H, C]), op=OP.mult)

            # ---- bias1 + ELU (batched over groups of blocks) ----
            GE = 8
            with tc.tile_pool(name="pe_sb", bufs=2) as pe:
                for g0 in range(0, NB, GE):
                    ng = min(GE, NB - g0)
                    view = h_sh[:, g0:g0 + ng, :]
                    nc.vector.tensor_tensor(
                        out=view, in0=view,
                        in1=b1r[:, None, :].to_broadcast([128, ng, HC]), op=OP.add)
                    negt = pe.tile([128, GE, HC], F32, tag="neg")
                    nc.vector.tensor_scalar(out=negt[:, :ng, :], in0=view, scalar1=0.0,
                                            scalar2=None, op0=OP.min)
                    expt = pe.tile([128, GE, HC], F32, tag="exp")
                    nc.scalar.activation(
                        out=expt[:, :ng, :].rearrange("p g c -> p (g c)"),
                        in_=negt[:, :ng, :].rearrange("p g c -> p (g c)"), func=AF.Exp)
                    nc.vector.tensor_scalar(out=view, in0=view, scalar1=0.0,
                                            scalar2=None, op0=OP.max)
                    nc.vector.tensor_tensor(out=view, in0=view, in1=expt[:, :ng, :], op=OP.add)
                    nc.vector.tensor_scalar(out=view, in0=view, scalar1=-1.0,
                                            scalar2=None, op0=OP.add)

            # ---- Phase C: layer-2 node transforms ----
            with (
                tc.tile_pool(name="pc_sb", bufs=3) as pc,
                tc.tile_pool(name="pc_ps", bufs=2, space="PSUM") as pcp,
            ):
                for i in range(NB):
                    hT = pc.tile([128, 2, 128], BF16, tag="hT")
                    for half in range(2):
                        tp = pcp.tile([128, 128], BF16, space="PSUM", tag="tp")
                        nc.tensor.transpose(
                            out=tp[:], in_=h_sh[:, i, half * 128:(half + 1) * 128],
                            identity=identb[:])
                        nc.scalar.copy(out=hT[:, half, :], in_=tp[:])
                    ps2l = pcp.tile([128, DOUT], F32, space="PSUM", tag="ps2l")
                    ps2r = pcp.tile([128, DOUT], F32, space="PSUM", tag="ps2r")
                    for half in range(2):
                        nc.tensor.matmul(out=ps2l[:], lhsT=hT[:, half, :], rhs=w2lb[:, half, :],
                                         start=(half == 0), stop=(half == 1))
                        nc.tensor.matmul(out=ps2r[:], lhsT=hT[:, half, :], rhs=w2rb[:, half, :],
                                         start=(half == 0), stop=(half == 1))
                    xl2t = pc.tile([128, DOUT], BF16, tag="xl2t")
                    nc.vector.tensor_tensor(out=xl2t[:], in0=ps2l[:], in1=bl2[:], op=OP.add)
                    nc.sync.dma_start(out=xl2_ag_in[i * 128:(i + 1) * 128, 0:DOUT], in_=xl2t[:])
                    nc.sync.dma_start(out=xl2_ag_in[i * 128:(i + 1) * 128, DOUT:2 * DOUT], in_=xl2t[:])
                    nc.vector.tensor_tensor(out=xr2_sh[:, i, :], in0=ps2r[:], in1=br2[:], op=OP.add)

            nc.gpsimd.collective_compute(
                "AllGather", OP.bypass, replica_groups=[list(range(NCORES))],
                ins=[xl2_ag_in[:].opt()], outs=[xl2_full[:].opt()],
            )

            # ---- Phase D: layer-2 edge loop (1 head, 64 ch, grouped) ----
            with (
                tc.tile_pool(name="pd_sb", bufs=2) as pd,
                tc.tile_pool(name="pd_s", bufs=(TMAX + 3) // 4 + 1) as pds,
                tc.tile_pool(name="pd_stp", bufs=2, space="PSUM") as pdstp,
                tc.tile_pool(name="pd_mp", bufs=2, space="PSUM") as pdmp,
                tc.tile_pool(name="pd_agg", bufs=2, space="PSUM") as pdagg,
            ):
                stcnt = [0]
                for b in range(NB):
                    tlo, thi, tb = int(T_LO[b]), int(T_HI[b]), int(TB[b])
                    c0 = int(dcol[b])
                    x2g = pd.tile([128, TMAX, 2 * DOUT], BF16, tag="x2g")
                    for (toff, nt, base) in ((0, tlo, 0), (tlo, thi, SPLIT)):
                        src_ap = xl2_full[:] if base == 0 else xl2_full[base:, :]
                        for t0 in range(0, nt, GATHER_TILES_PER_CALL):
                            ntc = min(GATHER_TILES_PER_CALL, nt - t0)
                            col0 = (c0 + toff + t0) * 8
                            nc.gpsimd.dma_gather(
                                out_ap=x2g[:, toff + t0:toff + t0 + ntc, :],
                                in_ap=src_ap,
                                idxs_ap=gidx_sb[:, col0:col0 + ntc * 8],
                                num_idxs=ntc * 128, num_idxs_reg=ntc * 128,
                                elem_size=2 * DOUT, queue_num=next_q(),
                            )
                    S_groups = []
                    ST_tiles = []
                    e_blk = pd.tile([128, TMAX], F32, tag="e2")
                    lr_blk = pd.tile([128, TMAX, DOUT], BF16, tag="lr2")
                    for g0 in range(0, tb, 4):
                        gs = min(4, tb - g0)
                        S4 = pds.tile([128, 4, 128], BF16, tag="S24")
                        S_groups.append(S4)
                        nc.vector.tensor_tensor(
                            out=S4[:, :gs, :],
                            in0=dstl_sb[:, c0 + g0:c0 + g0 + gs, None].to_broadcast([128, gs, 128]),
                            in1=iota[:, None, :].to_broadcast([128, gs, 128]),
                            op=OP.is_equal)
                        for j in range(gs):
                            stp = pdstp.tile([128, 128], BF16, space="PSUM", tag="stp2")
                            nc.tensor.transpose(out=stp[:], in_=S4[:, j, :], identity=identb[:])
                            ST = pd.tile([128, 128], BF16, tag=f"ST2{j % 2}")
                            ST_tiles.append(ST)
                            if stcnt[0] % 2 == 0:
                                nc.scalar.copy(out=ST[:], in_=stp[:])
                            else:
                                nc.vector.tensor_copy(out=ST[:], in_=stp[:])
                            stcnt[0] += 1
                        for j0 in range(0, gs, 2):
                            js = min(2, gs - j0)
                            mp = pdmp.tile([128, 2, 512], F32, space="PSUM", tag="mp2")
                            for j in range(js):
                                nc.tensor.matmul(out=mp[:, j, 0:DOUT], lhsT=ST_tiles[g0 + j0 + j][:],
                                                 rhs=xr2_sh[:, b, :], start=True, stop=False)
                            for j in range(js):
                                nc.tensor.matmul(out=mp[:, j, 0:DOUT], lhsT=identb[:],
                                                 rhs=x2g[:, g0 + j0 + j, 0:DOUT], start=False, stop=True)
                            nc.scalar.activation(
                                out=lr_blk[:, g0 + j0:g0 + j0 + js, :],
                                in_=mp[:, :js, 0:DOUT], func=AF.Prelu, alpha=NEG_SLOPE)
                        am4 = pd.tile([128, 4, DOUT], BF16, tag="am24")
                        nc.vector.tensor_tensor(
                            out=am4[:, :gs, :], in0=lr_blk[:, g0:g0 + gs, :],
                            in1=att2f[:, None, :].to_broadcast([128, gs, DOUT]), op=OP.mult)
                        nc.vector.tensor_reduce(
                            out=e_blk[:, g0:g0 + gs], in_=am4[:, :gs, :],
                            axis=mybir.AxisListType.X, op=OP.add)
                    exb = pd.tile([128, TMAX], BF16, tag="ex2")
                    nc.scalar.activation(out=exb[:, :tb], in_=e_blk[:, :tb], func=AF.Exp)
                    agg = pdagg.tile([128, DOUT + 1], F32, space="PSUM", tag="agg2")
                    for g0 in range(0, tb, 4):
                        gs = min(4, tb - g0)
                        S4 = S_groups[g0 // 4]
                        xlx4 = pd.tile([128, 4, DOUT + 1], BF16, tag="xlx24")
                        nc.vector.tensor_tensor(
                            out=xlx4[:, :gs, 0:DOUT], in0=x2g[:, g0:g0 + gs, 0:DOUT],
                            in1=exb[:, g0:g0 + gs, None].to_broadcast([128, gs, DOUT]),
                            op=OP.mult)
                        nc.vector.tensor_copy(out=xlx4[:, :gs, DOUT:DOUT + 1],
                                              in_=exb[:, g0:g0 + gs, None])
                        for j in range(gs):
                            t = g0 + j
                            nc.tensor.matmul(out=agg[:], lhsT=S4[:, j, :], rhs=xlx4[:, j, :],
                                             start=(t == 0), stop=(t == tb - 1))
                    rd = pd.tile([128, 1], F32, tag="rd2")
                    nc.vector.reciprocal(out=rd[:], in_=agg[:, DOUT:DOUT + 1])
                    nc.vector.tensor_tensor(
                        out=o1_sh[:, b, :], in0=agg[:, 0:DOUT],
                        in1=rd[:, 0:1].to_broadcast([128, DOUT]), op=OP.mult)

            # ---- Phase E: bias2 + outputs + log_softmax ----
            with tc.tile_pool(name="pf_sb", bufs=1) as pf:
                nc.vector.tensor_tensor(
                    out=o1_sh[:], in0=o1_sh[:],
                    in1=b2r[:, None, :].to_broadcast([128, NB, DOUT]), op=OP.add)
                nc.sync.dma_start(
                    out=out1.rearrange("(b p) c -> p b c", p=128), in_=o1_sh[:])
                rmax = pf.tile([128, NB], F32)
                nc.vector.tensor_reduce(out=rmax[:], in_=o1_sh[:],
                                        axis=mybir.AxisListType.X, op=OP.max)
                xm = pf.tile([128, NB, DOUT], F32)
                nc.vector.tensor_tensor(
                    out=xm[:], in0=o1_sh[:],
                    in1=rmax[:, :, None].to_broadcast([128, NB, DOUT]), op=OP.subtract)
                pexp = pf.tile([128, NB, DOUT], F32)
                nc.scalar.activation(out=pexp[:].rearrange("p b c -> p (b c)"),
                                     in_=xm[:].rearrange("p b c -> p (b c)"), func=AF.Exp)
                ssum = pf.tile([128, NB], F32)
                nc.vector.tensor_reduce(out=ssum[:], in_=pexp[:],
                                        axis=mybir.AxisListType.X, op=OP.add)
                lns = pf.tile([128, NB], F32)
                nc.scalar.activation(out=lns[:], in_=ssum[:], func=AF.Ln)
                nc.vector.tensor_tensor(
                    out=xm[:], in0=xm[:],
                    in1=lns[:, :, None].to_broadcast([128, NB, DOUT]), op=OP.subtract)
                nc.sync.dma_start(
                    out=out2.rearrange("(b p) c -> p b c", p=128), in_=xm[:])

    nc.compile()
    return nc


def kernel(x, edge_index, Wl1, bl1, Wr1, br1, att1, bias1,
           Wl2, bl2, Wr2, br2, att2, bias2):
    x = np.asarray(x, np.float32)
    edge_index = np.asarray(edge_index)
    P = _prep_host(x, edge_index)
    nc = _build_bass(P)

    rep = lambda v, w: np.tile(np.asarray(v, np.float32).reshape(1, -1), (128, 1))[:, :w]
    consts = {
        "iota": np.tile(np.arange(128, dtype=np.float32)[None, :], (128, 1)),
        "identb": np.eye(128).astype(ml_dtypes.bfloat16),
        "identf": np.eye(128, dtype=np.float32),
        "att1r": np.tile(np.asarray(att1, np.float32).reshape(1, HC), (128, 1)),
        "att2r": np.tile(np.asarray(att2, np.float32).reshape(1, DOUT), (128, 1)),
        "w1l": np.asarray(Wl1, np.float32), "w1r": np.asarray(Wr1, np.float32),
        "w2l": np.asarray(Wl2, np.float32), "w2r": np.asarray(Wr2, np.float32),
        "bl1r": rep(bl1, HC), "br1r": rep(br1, HC), "b1r": rep(bias1, HC),
        "bl2r": rep(bl2, DOUT), "br2r": rep(br2, DOUT), "b2r": rep(bias2, DOUT),
    }
    in_maps = []
    for c in range(NCORES):
        m = dict(consts)
        m["xT"] = P["xT_sh"][c]
        m["gidx"] = P["gidx"][c]
        m["dstl"] = P["dstl"][c]
        in_maps.append(m)

    trace = bool(os.environ.get("KERNEL_TRACE"))
    res = bass_utils.run_bass_kernel_spmd(
        nc, in_maps, core_ids=list(range(NCORES)), trace=trace)
    if trace and res.exec_time_ns:
        print(f"HW exec time: {res.exec_time_ns} ns")
        kernel.last_results = res

    counts = P["counts"]
    h = np.concatenate([res.results[c]["out1"][:counts[c]] for c in range(NCORES)], axis=0)
    ls = np.concatenate([res.results[c]["out2"][:counts[c]] for c in range(NCORES)], axis=0)
    return h, ls
